# revision 1
# baseline (speedup 1.0000x reference)
"""Trainium2 Bass kernel for nn_LinkPredictor (2-layer GCN + edge-dot decode).

Strategy (8 NeuronCores, SPMD):
  - Nodes sharded: core c owns rows [c*12544, (c+1)*12544) of the padded
    node table (N=100000 padded to 100352 = 8*98*128).
  - Edges assigned to the core owning their dst. Per core, edges are grouped
    by (dst window of 128 nodes, src chunk of 25088 rows) with a uniform slot
    budget B per group (padded with dead slots, norm=0) so all 8 cores run an
    identical program.
  - GCN norm (dinv[s]*dinv[d]) is folded into one-hot selection matrices S
    built on-device by a dual-op tensor_scalar: S[e,:] = (iota==dstloc[e])*norm[e].
  - Message gather: dma_gather (GPSIMD SWDGE, 4 queues) from a bf16 node table
    in DRAM; segment-sum = PE matmul accumulation S^T @ M into PSUM (fp32).
  - Self-loops handled separately (dinv^2 * h[n], no gather).
  - Inter-layer full-table exchange via AllGather collectives.
  - Decode: gather z2[s], z2[d], DVE multiply + reduce.
"""
import contextlib
import math
import numpy as np
import ml_dtypes

import concourse.bass as bass
import concourse.tile as tile
from concourse import bacc, mybir
from concourse.bass_utils import run_bass_kernel_spmd
from concourse.tile_rust import add_dep_helper

F32 = mybir.dt.float32
BF16 = mybir.dt.bfloat16
I16 = mybir.dt.int16
BF = ml_dtypes.bfloat16


class Cfg:
    def __init__(self, N=100000, E=1600000, EL=100000, D=128, ncores=8,
                 nw=98, nchunks=4, wb=4):
        self.N, self.E, self.EL, self.D, self.NC = N, E, EL, D, ncores
        self.NW = nw                      # windows (128 nodes each) per core
        self.SHARD = nw * 128             # nodes per core (padded)
        self.NP = self.SHARD * ncores     # padded node count
        assert self.NP >= N
        self.NCH = nchunks                # src chunks (int16 index range)
        assert self.NP % nchunks == 0
        self.CHROWS = self.NP // nchunks
        assert self.CHROWS <= 32768
        self.WB = wb                      # windows per gather/aggregate batch
        self.NBATCH = math.ceil(nw / wb)


DEFAULT = Cfg()


def _wrap_idxs(idx):
    """[n] ints -> [128, n//16] int16 wrapped in 16 partitions, replicated 8x."""
    n = len(idx)
    assert n % 16 == 0
    w = np.asarray(idx, dtype=np.int16).reshape(n // 16, 16).T
    return np.ascontiguousarray(np.tile(w, (8, 1)))


def host_prep(cfg, x, edge_index, edge_label_index, W1, b1, W2, b2):
    """All host-side sharding/layout. Returns (per-core input maps, meta)."""
    c = cfg
    src = np.asarray(edge_index[0], dtype=np.int64)
    dst = np.asarray(edge_index[1], dtype=np.int64)
    deg = np.bincount(dst, minlength=c.N).astype(np.float64) + 1.0
    dinv = 1.0 / np.sqrt(deg)                      # [N]
    dinv_p = np.ones(c.NP, dtype=np.float64)
    dinv_p[:c.N] = dinv
    norm_e = (dinv[src] * dinv[dst]).astype(np.float32)

    core_of = dst // c.SHARD
    w_of = (dst - core_of * c.SHARD) // 128
    ch_of = src // c.CHROWS

    key = (core_of * c.NW + w_of) * c.NCH + ch_of
    order = np.argsort(key, kind="stable")
    ngroups = c.NC * c.NW * c.NCH
    counts = np.bincount(key[order], minlength=ngroups)
    B = int(128 * math.ceil(max(int(counts.max()), 1) / 128))
    starts = np.zeros(ngroups + 1, dtype=np.int64)
    np.cumsum(counts, out=starts[1:])

    TPG = B // 128                       # tiles per group
    TOT = c.NW * c.NCH * B               # slots per core per layer
    TOT_TILES = TOT // 128

    # global slot order per core: batch b -> chunk ch -> window w (in batch) -> i
    idx_arr = np.zeros((c.NC, TOT), dtype=np.int64)
    dstloc_arr = np.zeros((c.NC, TOT), dtype=np.float32)
    norm_arr = np.zeros((c.NC, TOT), dtype=np.float32)
    for core in range(c.NC):
        pos = 0
        for b in range(c.NBATCH):
            wlo, whi = b * c.WB, min((b + 1) * c.WB, c.NW)
            for ch in range(c.NCH):
                for w in range(wlo, whi):
                    g = (core * c.NW + w) * c.NCH + ch
                    eids = order[starts[g]:starts[g + 1]]
                    n = len(eids)
                    idx_arr[core, pos:pos + n] = src[eids] - ch * c.CHROWS
                    dstloc_arr[core, pos:pos + n] = (
                        dst[eids] - core * c.SHARD - w * 128)
                    norm_arr[core, pos:pos + n] = norm_e[eids]
                    pos += B
        assert pos == TOT

    # decode: label edge j -> core j // ELC; slots grouped by (chunk(s), chunk(d))
    assert c.EL % c.NC == 0
    ELC = c.EL // c.NC
    ls = np.asarray(edge_label_index[0], dtype=np.int64)
    ld = np.asarray(edge_label_index[1], dtype=np.int64)
    kd = (ls // c.CHROWS) * c.NCH + (ld // c.CHROWS)
    NG_DEC = c.NCH * c.NCH
    B_dec = 0
    for core in range(c.NC):
        cnt = np.bincount(kd[core * ELC:(core + 1) * ELC], minlength=NG_DEC)
        B_dec = max(B_dec, int(cnt.max()))
    B_dec = 128 * math.ceil(max(B_dec, 1) / 128)
    TOT_DEC = NG_DEC * B_dec
    idx_s = np.zeros((c.NC, TOT_DEC), dtype=np.int64)
    idx_d = np.zeros((c.NC, TOT_DEC), dtype=np.int64)
    slot2j = np.full((c.NC, TOT_DEC), -1, dtype=np.int64)
    for core in range(c.NC):
        jlo = core * ELC
        kk = kd[jlo:jlo + ELC]
        o = np.argsort(kk, kind="stable")
        cnt = np.bincount(kk, minlength=NG_DEC)
        st = np.zeros(NG_DEC + 1, dtype=np.int64)
        np.cumsum(cnt, out=st[1:])
        for g in range(NG_DEC):
            js = o[st[g]:st[g + 1]] + jlo
            n = len(js)
            pos = g * B_dec
            idx_s[core, pos:pos + n] = ls[js] - (g // c.NCH) * c.CHROWS
            idx_d[core, pos:pos + n] = ld[js] - (g % c.NCH) * c.CHROWS
            slot2j[core, pos:pos + n] = js

    xp = np.zeros((c.NP, c.D), dtype=np.float32)
    xp[:c.N] = np.asarray(x, dtype=np.float32)
    dinv_f = dinv_p.astype(np.float32)
    in_maps = []
    for core in range(c.NC):
        sl = slice(core * c.SHARD, (core + 1) * c.SHARD)
        in_maps.append({
            "xT": np.ascontiguousarray(xp[sl].T).astype(BF),
            "W1": np.asarray(W1, dtype=np.float32).astype(BF),
            "W2": np.asarray(W2, dtype=np.float32).astype(BF),
            "b1r": np.tile(np.asarray(b1, np.float32)[None, :], (128, 1)),
            "b2r": np.tile(np.asarray(b2, np.float32)[None, :], (128, 1)),
            "dinv2": np.ascontiguousarray(
                (dinv_f[sl] ** 2).reshape(c.NW, 128).T),
            "gidx": _wrap_idxs(idx_arr[core]),
            "dstloc": np.ascontiguousarray(
                dstloc_arr[core].reshape(TOT_TILES, 128).T),
            "gnorm": np.ascontiguousarray(
                norm_arr[core].reshape(TOT_TILES, 128).T),
            "didx_s": _wrap_idxs(idx_s[core]),
            "didx_d": _wrap_idxs(idx_d[core]),
        })
    meta = dict(B=B, TPG=TPG, TOT=TOT, TOT_TILES=TOT_TILES,
                B_dec=B_dec, TOT_DEC=TOT_DEC, slot2j=slot2j)
    return in_maps, meta


def build_program(cfg, meta, num_cores=None):
    c = cfg
    NCores = num_cores or c.NC
    B, TPG, TOT, TOT_TILES = meta["B"], meta["TPG"], meta["TOT"], meta["TOT_TILES"]
    B_dec, TOT_DEC = meta["B_dec"], meta["TOT_DEC"]
    D = c.D

    nc = bacc.Bacc("TRN2", target_bir_lowering=False, debug=False,
                   num_devices=NCores, num_swdge_queues=min(4, c.NCH))
    NQ = min(4, c.NCH)

    xT_in = nc.dram_tensor("xT", [D, c.SHARD], BF16, kind="ExternalInput")
    W1_in = nc.dram_tensor("W1", [D, D], BF16, kind="ExternalInput")
    W2_in = nc.dram_tensor("W2", [D, D], BF16, kind="ExternalInput")
    b1_in = nc.dram_tensor("b1r", [128, D], F32, kind="ExternalInput")
    b2_in = nc.dram_tensor("b2r", [128, D], F32, kind="ExternalInput")
    dinv2_in = nc.dram_tensor("dinv2", [128, c.NW], F32, kind="ExternalInput")
    gidx_in = nc.dram_tensor("gidx", [128, TOT // 16], I16, kind="ExternalInput")
    dstloc_in = nc.dram_tensor("dstloc", [128, TOT_TILES], F32, kind="ExternalInput")
    gnorm_in = nc.dram_tensor("gnorm", [128, TOT_TILES], F32, kind="ExternalInput")
    didx_s_in = nc.dram_tensor("didx_s", [128, TOT_DEC // 16], I16, kind="ExternalInput")
    didx_d_in = nc.dram_tensor("didx_d", [128, TOT_DEC // 16], I16, kind="ExternalInput")
    dots_out = nc.dram_tensor("dots", [128, TOT_DEC // 128], F32, kind="ExternalOutput")

    gst = {"count": 0, "prev": None}

    def emit_gather(out_ap, in_ap, idx_ap, n_idx):
        q = gst["count"] % NQ
        inst = nc.gpsimd.dma_gather(out_ap, in_ap, idx_ap, n_idx, n_idx, D,
                                    queue_num=q, single_packet=False)
        if gst["prev"] is not None:
            add_dep_helper(inst.ins, gst["prev"].ins, sync=False,
                           reason="pin swdge queue order")
        gst["prev"] = inst
        gst["count"] += 1
        return inst

    shard1 = nc.dram_tensor("shard1", [c.SHARD, D], BF16)
    shard2 = nc.dram_tensor("shard2", [c.SHARD, D], BF16)
    shardz = nc.dram_tensor("shardz", [c.SHARD, D], BF16)
    table1 = nc.dram_tensor("table1", [c.NP, D], BF16)
    table2 = nc.dram_tensor("table2", [c.NP, D], BF16)
    tablez = nc.dram_tensor("tablez", [c.NP, D], BF16)

    iota_dram = nc.inline_tensor(
        np.tile(np.arange(128, dtype=np.float32), (128, 1)).astype(BF), "iota_c")
    ident_dram = nc.inline_tensor(np.eye(128, dtype=np.float32).astype(BF), "ident_c")

    cc_sem = nc.alloc_semaphore("cc_sem")
    core_ids = list(range(NCores))

    with tile.TileContext(nc) as tc:
        with contextlib.ExitStack() as es:
            const = es.enter_context(tc.tile_pool(name="const", bufs=1))
            meta_p = es.enter_context(tc.tile_pool(name="meta", bufs=1))

            w1_sb = const.tile([D, D], BF16); nc.sync.dma_start(w1_sb[:], W1_in[:])
            w2_sb = const.tile([D, D], BF16); nc.sync.dma_start(w2_sb[:], W2_in[:])
            b1_sb = const.tile([128, D], F32); nc.sync.dma_start(b1_sb[:], b1_in[:])
            b2_sb = const.tile([128, D], F32); nc.sync.dma_start(b2_sb[:], b2_in[:])
            dinv2_sb = const.tile([128, c.NW], F32)
            nc.sync.dma_start(dinv2_sb[:], dinv2_in[:])
            iota_sb = const.tile([128, 128], BF16)
            nc.sync.dma_start(iota_sb[:], iota_dram[:])
            ident_sb = const.tile([128, 128], BF16)
            nc.sync.dma_start(ident_sb[:], ident_dram[:])
            gidx_sb = meta_p.tile([128, TOT // 16], I16)
            nc.sync.dma_start(gidx_sb[:], gidx_in[:])
            dstloc_sb = meta_p.tile([128, TOT_TILES], F32)
            nc.sync.dma_start(dstloc_sb[:], dstloc_in[:])
            gnorm_sb = meta_p.tile([128, TOT_TILES], F32)
            nc.sync.dma_start(gnorm_sb[:], gnorm_in[:])

            def all_gather(shard, table_out, sem, n_before):
                tc.strict_bb_all_engine_barrier()
                with tc.tile_critical():
                    nc.gpsimd.collective_compute(
                        "AllGather", mybir.AluOpType.bypass,
                        replica_groups=[core_ids],
                        ins=[shard[:]], outs=[table_out[:]],
                    ).then_inc(cc_sem)
                    nc.gpsimd.wait_ge(cc_sem, n_before + 1)
                tc.strict_bb_all_engine_barrier()

            def layer(lid, table, h_tiles, bias_sb, shard_next, sem_next,
                      out_pool, make_next):
                out_tiles = []
                with tc.tile_pool(name=f"M{lid}", bufs=2) as Mp, \
                     tc.tile_pool(name=f"S{lid}", bufs=4) as Sp, \
                     tc.tile_pool(name=f"ag{lid}", bufs=4, space="PSUM") as agp, \
                     tc.tile_pool(name=f"tp{lid}", bufs=2, space="PSUM") as tpp, \
                     tc.tile_pool(name=f"ep{lid}", bufs=3) as epp:
                    for b in range(c.NBATCH):
                        wlo = b * c.WB
                        whi = min(wlo + c.WB, c.NW)
                        nwb = whi - wlo
                        cols_per_ch = nwb * TPG
                        Mt = Mp.tile([128, c.NCH * cols_per_ch, D], BF16, tag="M")
                        slot_base = wlo * c.NCH * B
                        for ch in range(c.NCH):
                            n_idx = nwb * B
                            off16 = (slot_base + ch * n_idx) // 16
                            emit_gather(
                                Mt[:, ch * cols_per_ch:(ch + 1) * cols_per_ch, :],
                                table[ch * c.CHROWS:(ch + 1) * c.CHROWS, :],
                                gidx_sb[:, off16:off16 + n_idx // 16],
                                n_idx)
                        tile_base = slot_base // 128
                        for wi in range(nwb):
                            w = wlo + wi
                            ps = agp.tile([128, D], F32, tag="agg")
                            nmm = c.NCH * TPG
                            k = 0
                            for ch in range(c.NCH):
                                for t in range(TPG):
                                    tcol = tile_base + (ch * nwb + wi) * TPG + t
                                    S = Sp.tile([128, 128], BF16, tag="S")
                                    nc.vector.tensor_scalar(
                                        S[:], iota_sb[:],
                                        dstloc_sb[:, tcol:tcol + 1],
                                        gnorm_sb[:, tcol:tcol + 1],
                                        mybir.AluOpType.is_equal,
                                        mybir.AluOpType.mult)
                                    mcol = (ch * nwb + wi) * TPG + t
                                    nc.tensor.matmul(
                                        ps[:], lhsT=S[:], rhs=Mt[:, mcol, :],
                                        start=(k == 0), stop=(k == nmm - 1))
                                    k += 1
                            selfz = epp.tile([128, D], F32, tag="selfz")
                            nc.vector.tensor_scalar(
                                selfz[:], h_tiles[w][:],
                                dinv2_sb[:, w:w + 1], None,
                                mybir.AluOpType.mult)
                            s1 = epp.tile([128, D], F32, tag="s1")
                            nc.vector.tensor_tensor(
                                s1[:], ps[:], selfz[:], op=mybir.AluOpType.add)
                            s2 = epp.tile([128, D], F32, tag="s2")
                            nc.vector.tensor_tensor(
                                s2[:], s1[:], bias_sb[:], op=mybir.AluOpType.add)
                            if make_next:
                                z = epp.tile([128, D], BF16, tag="z")
                                nc.scalar.activation(
                                    z[:], s2[:], mybir.ActivationFunctionType.Relu)
                                zt_ps = tpp.tile([128, D], BF16, tag="zt")
                                nc.tensor.transpose(zt_ps[:], z[:], ident_sb[:])
                                zT = epp.tile([128, D], BF16, tag="zT")
                                nc.vector.tensor_copy(zT[:], zt_ps[:])
                                h2ps = tpp.tile([128, D], F32, tag="h2")
                                nc.tensor.matmul(h2ps[:], lhsT=zT[:], rhs=w2_sb[:],
                                                 start=True, stop=True)
                                ht = out_pool.tile([128, D], BF16, tag="nxt")
                                nc.vector.tensor_copy(ht[:], h2ps[:])
                            else:
                                ht = out_pool.tile([128, D], BF16, tag="nxt")
                                nc.scalar.activation(
                                    ht[:], s2[:], mybir.ActivationFunctionType.Relu)
                            nc.sync.dma_start(
                                shard_next[w * 128:(w + 1) * 128, :], ht[:])
                            out_tiles.append(ht)
                return out_tiles

            with tc.tile_pool(name="hsb2", bufs=c.NW) as hsb2:
                with tc.tile_pool(name="hsb1", bufs=c.NW) as hsb1:
                    # P0: h1 = x @ W1 for own shard
                    h1_tiles = []
                    with tc.tile_pool(name="p0", bufs=3) as p0, \
                         tc.tile_pool(name="p0ps", bufs=2, space="PSUM") as p0ps:
                        for w in range(c.NW):
                            xt = p0.tile([D, 128], BF16)
                            nc.sync.dma_start(
                                xt[:], xT_in[:, w * 128:(w + 1) * 128])
                            ps = p0ps.tile([128, D], F32, tag="ps")
                            nc.tensor.matmul(ps[:], lhsT=xt[:], rhs=w1_sb[:],
                                             start=True, stop=True)
                            h1t = hsb1.tile([128, D], BF16, tag="h1t")
                            nc.vector.tensor_copy(h1t[:], ps[:])
                            nc.sync.dma_start(
                                shard1[w * 128:(w + 1) * 128, :], h1t[:])
                            h1_tiles.append(h1t)
                    all_gather(shard1, table1, None, 0)
                    h2_tiles = layer(1, table1, h1_tiles, b1_sb, shard2,
                                     None, hsb2, make_next=True)
                all_gather(shard2, table2, None, 1)
                with tc.tile_pool(name="zsink", bufs=3) as zsink:
                    layer(2, table2, h2_tiles, b2_sb, shardz,
                          None, zsink, make_next=False)
            all_gather(shardz, tablez, None, 2)

            # decode
            with tc.tile_pool(name="didx", bufs=1) as didxp, \
                 tc.tile_pool(name="dM", bufs=1) as dMp, \
                 tc.tile_pool(name="dw", bufs=4) as dwp, \
                 tc.tile_pool(name="dout", bufs=1) as doutp:
                ds_sb = didxp.tile([128, TOT_DEC // 16], I16)
                nc.sync.dma_start(ds_sb[:], didx_s_in[:])
                dd_sb = didxp.tile([128, TOT_DEC // 16], I16)
                nc.sync.dma_start(dd_sb[:], didx_d_in[:])
                Ms = dMp.tile([128, TOT_DEC // 128, D], BF16, tag="Ms")
                Md = dMp.tile([128, TOT_DEC // 128, D], BF16, tag="Md")
                res = doutp.tile([128, TOT_DEC // 128], F32)
                NG_DEC = c.NCH * c.NCH
                for g in range(NG_DEC):
                    chs, chd = g // c.NCH, g % c.NCH
                    off16 = g * B_dec // 16
                    coff = g * B_dec // 128
                    ncols = B_dec // 128
                    emit_gather(
                        Ms[:, coff:coff + ncols, :],
                        tablez[chs * c.CHROWS:(chs + 1) * c.CHROWS, :],
                        ds_sb[:, off16:off16 + B_dec // 16], B_dec)
                    emit_gather(
                        Md[:, coff:coff + ncols, :],
                        tablez[chd * c.CHROWS:(chd + 1) * c.CHROWS, :],
                        dd_sb[:, off16:off16 + B_dec // 16], B_dec)
                for col in range(TOT_DEC // 128):
                    mm = dwp.tile([128, D], F32, tag="mm")
                    nc.vector.tensor_tensor(
                        mm[:], Ms[:, col, :], Md[:, col, :],
                        op=mybir.AluOpType.mult)
                    nc.vector.reduce_sum(res[:, col:col + 1], mm[:],
                                         axis=mybir.AxisListType.X)
                nc.sync.dma_start(dots_out[:], res[:])

    nc.compile()
    return nc


def assemble_output(cfg, meta, results):
    c = cfg
    slot2j = meta["slot2j"]
    out = np.zeros(c.EL, dtype=np.float32)
    for core in range(len(results)):
        d = np.asarray(results[core]["dots"], dtype=np.float32)
        flat = d.T.reshape(-1)             # slot i -> d[i%128, i//128]
        s2j = slot2j[core]
        valid = s2j >= 0
        out[s2j[valid]] = flat[valid]
    return out


def run_pipeline(x, edge_index, edge_label_index, W1, b1, W2, b2,
                 cfg=None, trace=False, tmpdir=None):
    cfg = cfg or DEFAULT
    in_maps, meta = host_prep(cfg, x, edge_index, edge_label_index,
                              W1, b1, W2, b2)
    nc = build_program(cfg, meta)
    res = run_bass_kernel_spmd(nc, in_maps, list(range(cfg.NC)),
                               trace=trace, tmpdir=tmpdir)
    return assemble_output(cfg, meta, res.results), res


def kernel(x, edge_index, edge_label_index, W1, b1, W2, b2):
    out, _ = run_pipeline(x, edge_index, edge_label_index, W1, b1, W2, b2)
    return out



# revision 12
# speedup vs baseline: 1.2624x; 1.2624x over previous
"""Trainium2 Bass kernel for nn_LinkPredictor (2-layer GCN + edge-dot decode).

Strategy (8 NeuronCores, SPMD), v2:
  - Nodes sharded: core c owns rows [c*12544, (c+1)*12544) of the padded
    node table (N=100000 padded to 100352 = 8*98*128).
  - dinv folded into node features: table rows hold hs = dinv[n] * (prev @ W),
    so per-edge norm disappears; output z = relu(dinv[v]*(agg + hs[v]) + b).
  - Edges grouped by (dst window of 128 nodes, src band of 3136 shard rows);
    per-group tile budget = max over cores (data-dependent, less padding).
  - Aggregation: pure one-hot S built by single-op DVE tensor_scalar
    (is_equal vs iota; dead slots dstloc=-1) or on the Scalar engine
    (Abs then Relu(1-a)); segment-sum = PE matmul accumulation into PSUM.
    Self-loop = identity matmul of hs; bias = rank-1 matmul (skipped if 0).
  - Message gather: dma_gather (SWDGE, 4 queues) from bf16 band tables,
    one gather per (window-batch, band) to amortize fixed cost.
  - Inter-layer exchange: 4 banded AllGather collectives per layer,
    pipelined with compute via dep edges (no global barriers).
  - Decode: gathers z[s], z[d] by band pair, DVE multiply + ACT accum row-sum.
"""
import contextlib
import math
import numpy as np
import ml_dtypes

import concourse.bass as bass
import concourse.tile as tile
from concourse import bacc, mybir
from concourse.bass_utils import run_bass_kernel_spmd
from concourse.tile_rust import add_dep_helper

F32 = mybir.dt.float32
BF16 = mybir.dt.bfloat16
I16 = mybir.dt.int16
BF = ml_dtypes.bfloat16
ACTF = mybir.ActivationFunctionType


class Cfg:
    def __init__(self, N=100000, E=1600000, EL=100000, D=128, ncores=8,
                 nw=98, nbands=4, wb=4, act_tenths=3):
        self.N, self.E, self.EL, self.D, self.NC = N, E, EL, D, ncores
        self.NW = nw                      # windows (128 nodes each) per core
        self.SHARD = nw * 128             # nodes per core (padded)
        self.NP = self.SHARD * ncores     # padded node count
        assert self.NP >= N
        self.NB = nbands                  # src bands per shard
        assert self.SHARD % nbands == 0
        self.BROWS = self.SHARD // nbands # shard rows per band
        self.TBROWS = self.BROWS * ncores # band table rows (gather idx range)
        assert self.TBROWS <= 32768
        self.WB = wb                      # windows per gather/aggregate batch
        self.NBATCH = math.ceil(nw / wb)
        self.ACT_TENTHS = act_tenths      # S-build tiles offloaded to ScalarE /10


DEFAULT = Cfg()


def _wrap_idxs(idx):
    """[n] ints -> [128, n//16] int16 wrapped in 16 partitions, replicated 8x."""
    n = len(idx)
    assert n % 16 == 0
    w = np.asarray(idx, dtype=np.int16).reshape(n // 16, 16).T
    return np.ascontiguousarray(np.tile(w, (8, 1)))


def _iter_tiles(cfg, T):
    """Yield (b, k, w, t, tcol) in the canonical slot/tile order."""
    tcol = 0
    for b in range(cfg.NBATCH):
        wlo, whi = b * cfg.WB, min((b + 1) * cfg.WB, cfg.NW)
        for k in range(cfg.NB):
            for w in range(wlo, whi):
                for t in range(int(T[w, k])):
                    yield b, k, w, t, tcol
                    tcol += 1


def host_prep(cfg, x, edge_index, edge_label_index, W1, b1, W2, b2):
    """All host-side sharding/layout. Returns (per-core input maps, meta)."""
    c = cfg
    src = np.asarray(edge_index[0], dtype=np.int64)
    dst = np.asarray(edge_index[1], dtype=np.int64)
    deg = np.bincount(dst, minlength=c.N).astype(np.float64) + 1.0
    dinv = 1.0 / np.sqrt(deg)                      # [N]
    dinv_p = np.ones(c.NP, dtype=np.float64)
    dinv_p[:c.N] = dinv
    dinv_f = dinv_p.astype(np.float32)

    # band table index of a node id
    def bidx_of(n):
        return (n // c.SHARD) * c.BROWS + (n % c.SHARD) % c.BROWS

    band_src = (src % c.SHARD) // c.BROWS
    bidx_src = bidx_of(src)
    core_of = dst // c.SHARD
    w_of = (dst % c.SHARD) // 128
    dloc = dst % 128

    key = (core_of * c.NW + w_of) * c.NB + band_src
    ngroups = c.NC * c.NW * c.NB
    order = np.argsort(key, kind="stable")
    counts = np.bincount(key, minlength=ngroups).reshape(c.NC, c.NW, c.NB)
    starts = np.zeros(ngroups + 1, dtype=np.int64)
    np.cumsum(np.bincount(key, minlength=ngroups), out=starts[1:])

    T = np.ceil(counts.max(axis=0) / 128).astype(np.int64)     # [NW, NB]
    TOT_TILES = int(T.sum())
    # per (b, k): number of tiles in that gather span
    span_tiles = np.zeros((c.NBATCH, c.NB), dtype=np.int64)
    for b in range(c.NBATCH):
        wlo, whi = b * c.WB, min((b + 1) * c.WB, c.NW)
        for k in range(c.NB):
            span_tiles[b, k] = T[wlo:whi, k].sum()
    TOT = TOT_TILES * 128

    idx_arr = np.zeros((c.NC, TOT), dtype=np.int64)
    dloc_arr = np.full((c.NC, TOT), -1.0, dtype=np.float32)
    for core in range(c.NC):
        pos = 0
        for b in range(c.NBATCH):
            wlo, whi = b * c.WB, min((b + 1) * c.WB, c.NW)
            for k in range(c.NB):
                for w in range(wlo, whi):
                    g = (core * c.NW + w) * c.NB + k
                    eids = order[starts[g]:starts[g + 1]]
                    n = len(eids)
                    idx_arr[core, pos:pos + n] = bidx_src[eids]
                    dloc_arr[core, pos:pos + n] = dloc[eids]
                    pos += int(T[w, k]) * 128
        assert pos == TOT

    # decode: label edge j -> core j // ELC; groups by (band(s), band(d))
    assert c.EL % c.NC == 0
    ELC = c.EL // c.NC
    ls = np.asarray(edge_label_index[0], dtype=np.int64)
    ld = np.asarray(edge_label_index[1], dtype=np.int64)
    ks = (ls % c.SHARD) // c.BROWS
    kd = (ld % c.SHARD) // c.BROWS
    gdec = ks * c.NB + kd
    NG_DEC = c.NB * c.NB
    cnt_dec = np.zeros((c.NC, NG_DEC), dtype=np.int64)
    for core in range(c.NC):
        cnt_dec[core] = np.bincount(gdec[core * ELC:(core + 1) * ELC],
                                    minlength=NG_DEC)
    Tdec = np.ceil(cnt_dec.max(axis=0) / 128).astype(np.int64)   # [NG_DEC]
    # process groups ordered by max(ks, kd) so decode pipelines with z gathers
    gorder = sorted(range(NG_DEC), key=lambda g: (max(g // c.NB, g % c.NB), g))
    TOT_DEC = int(Tdec.sum()) * 128
    idx_s = np.zeros((c.NC, TOT_DEC), dtype=np.int64)
    idx_d = np.zeros((c.NC, TOT_DEC), dtype=np.int64)
    slot2j = np.full((c.NC, TOT_DEC), -1, dtype=np.int64)
    bidx_s = bidx_of(ls)
    bidx_d = bidx_of(ld)
    for core in range(c.NC):
        jlo = core * ELC
        kk = gdec[jlo:jlo + ELC]
        o = np.argsort(kk, kind="stable")
        st = np.zeros(NG_DEC + 1, dtype=np.int64)
        np.cumsum(np.bincount(kk, minlength=NG_DEC), out=st[1:])
        pos = 0
        for g in gorder:
            js = o[st[g]:st[g + 1]] + jlo
            n = len(js)
            idx_s[core, pos:pos + n] = bidx_s[js]
            idx_d[core, pos:pos + n] = bidx_d[js]
            slot2j[core, pos:pos + n] = js
            pos += int(Tdec[g]) * 128
        assert pos == TOT_DEC

    xp = np.zeros((c.NP, c.D), dtype=np.float32)
    xp[:c.N] = np.asarray(x, dtype=np.float32)
    use_b1 = bool(np.any(np.asarray(b1)))
    use_b2 = bool(np.any(np.asarray(b2)))

    in_maps = []
    for core in range(c.NC):
        sl = slice(core * c.SHARD, (core + 1) * c.SHARD)
        dsh = dinv_f[sl]
        m = {
            "xT": np.ascontiguousarray(xp[sl].T).astype(BF),
            "W1": np.asarray(W1, dtype=np.float32).astype(BF),
            "W2": np.asarray(W2, dtype=np.float32).astype(BF),
            "dinv": np.ascontiguousarray(dsh.reshape(c.NW, 128).T),
            "gidx": _wrap_idxs(idx_arr[core]),
            "dstloc": np.ascontiguousarray(
                dloc_arr[core].reshape(TOT_TILES, 128).T),
            "negdst": np.ascontiguousarray(
                (-dloc_arr[core]).reshape(TOT_TILES, 128).T),
            "didx_s": _wrap_idxs(idx_s[core]),
            "didx_d": _wrap_idxs(idx_d[core]),
        }
        if use_b1 or use_b2:
            m["b1r"] = np.asarray(b1, np.float32)[None, :].astype(BF)
            m["b2r"] = np.asarray(b2, np.float32)[None, :].astype(BF)
            m["invd"] = (1.0 / dsh)[None, :].astype(BF)
        in_maps.append(m)
    meta = dict(T=T, span_tiles=span_tiles, TOT=TOT, TOT_TILES=TOT_TILES,
                Tdec=Tdec, gorder=gorder, TOT_DEC=TOT_DEC, slot2j=slot2j,
                use_b1=use_b1, use_b2=use_b2)
    return in_maps, meta


def build_program(cfg, meta, num_cores=None):
    c = cfg
    NCores = num_cores or c.NC
    T, span_tiles = meta["T"], meta["span_tiles"]
    TOT, TOT_TILES = meta["TOT"], meta["TOT_TILES"]
    Tdec, gorder, TOT_DEC = meta["Tdec"], meta["gorder"], meta["TOT_DEC"]
    use_b = {1: meta["use_b1"], 2: meta["use_b2"]}
    D = c.D
    TBMAX = int(span_tiles.sum(axis=1).max())

    nc = bacc.Bacc("TRN2", target_bir_lowering=False, debug=False,
                   num_devices=NCores, num_swdge_queues=4)
    NQ = 4

    xT_in = nc.dram_tensor("xT", [D, c.SHARD], BF16, kind="ExternalInput")
    W1_in = nc.dram_tensor("W1", [D, D], BF16, kind="ExternalInput")
    W2_in = nc.dram_tensor("W2", [D, D], BF16, kind="ExternalInput")
    dinv_in = nc.dram_tensor("dinv", [128, c.NW], F32, kind="ExternalInput")
    gidx_in = nc.dram_tensor("gidx", [128, TOT // 16], I16, kind="ExternalInput")
    dstloc_in = nc.dram_tensor("dstloc", [128, TOT_TILES], F32, kind="ExternalInput")
    negdst_in = nc.dram_tensor("negdst", [128, TOT_TILES], F32, kind="ExternalInput")
    didx_s_in = nc.dram_tensor("didx_s", [128, TOT_DEC // 16], I16, kind="ExternalInput")
    didx_d_in = nc.dram_tensor("didx_d", [128, TOT_DEC // 16], I16, kind="ExternalInput")
    if use_b[1] or use_b[2]:
        b1_in = nc.dram_tensor("b1r", [1, D], BF16, kind="ExternalInput")
        b2_in = nc.dram_tensor("b2r", [1, D], BF16, kind="ExternalInput")
        invd_in = nc.dram_tensor("invd", [1, c.SHARD], BF16, kind="ExternalInput")
    dots_out = nc.dram_tensor("dots", [128, TOT_DEC // 128], F32, kind="ExternalOutput")

    # per-band DRAM shards and gathered tables (layer 1, layer 2, z)
    shard_b = {l: [nc.dram_tensor(f"shard{l}_{k}", [c.BROWS, D], BF16)
                   for k in range(c.NB)] for l in (1, 2, 3)}
    table_b = {l: [nc.dram_tensor(f"table{l}_{k}", [c.TBROWS, D], BF16,
                                  addr_space="Shared")
                   for k in range(c.NB)] for l in (1, 2, 3)}

    iota_dram = nc.inline_tensor(
        np.tile(np.arange(128, dtype=np.float32), (128, 1)).astype(BF), "iota_c")
    ident_dram = nc.inline_tensor(np.eye(128, dtype=np.float32).astype(BF), "ident_c")

    core_ids = list(range(NCores))
    gst = {"count": 0, "prev": None}
    ccst = {"inst": {}}                 # (l, k) -> collective instruction

    def emit_gather(out_ap, in_ap, idx_ap, n_idx, dep_cc=None):
        q = gst["count"] % NQ
        inst = nc.gpsimd.dma_gather(out_ap, in_ap, idx_ap, n_idx, n_idx, D,
                                    queue_num=q, single_packet=False)
        if gst["prev"] is not None:
            add_dep_helper(inst.ins, gst["prev"].ins, sync=False,
                           reason="pin swdge queue order")
        if dep_cc is not None:
            add_dep_helper(inst.ins, dep_cc.ins, sync=True,
                           reason="gather after collective")
        gst["prev"] = inst
        gst["count"] += 1
        return inst

    def emit_collective(l, k, dep_dmas):
        cc = nc.gpsimd.collective_compute(
            "AllGather", mybir.AluOpType.bypass,
            replica_groups=[core_ids],
            ins=[shard_b[l][k][:]], outs=[table_b[l][k][:]],
        )
        # input-side ordering (shard writes -> collective) is handled by
        # tile's annotate_comm_input_writers; dep_dmas kept for reference
        _ = dep_dmas
        if gst["prev"] is not None:
            add_dep_helper(cc.ins, gst["prev"].ins, sync=False,
                           reason="order on gpsimd")
        gst["prev"] = cc
        ccst["inst"][(l, k)] = cc
        return cc

    def write_shard(l, w, ht):
        """DMA window w rows of tile ht into the banded shard tensors."""
        insts = []
        lo, hi = w * 128, (w + 1) * 128
        k0, k1 = lo // c.BROWS, (hi - 1) // c.BROWS
        for k in range(k0, k1 + 1):
            rlo, rhi = max(lo, k * c.BROWS), min(hi, (k + 1) * c.BROWS)
            i = nc.sync.dma_start(
                shard_b[l][k][rlo - k * c.BROWS: rhi - k * c.BROWS, :],
                ht[rlo - lo: rhi - lo, :])
            insts.append((k, i))
        return insts

    with tile.TileContext(nc) as tc:
        with contextlib.ExitStack() as es:
            const = es.enter_context(tc.tile_pool(name="const", bufs=1))
            meta_p = es.enter_context(tc.tile_pool(name="meta", bufs=1))

            w1_sb = const.tile([D, D], BF16); nc.sync.dma_start(w1_sb[:], W1_in[:])
            w2_sb = const.tile([D, D], BF16); nc.sync.dma_start(w2_sb[:], W2_in[:])
            dinv_sb = const.tile([128, c.NW], F32)
            nc.sync.dma_start(dinv_sb[:], dinv_in[:])
            iota_sb = const.tile([128, 128], BF16)
            nc.sync.dma_start(iota_sb[:], iota_dram[:])
            ident_sb = const.tile([128, 128], BF16)
            nc.sync.dma_start(ident_sb[:], ident_dram[:])
            if use_b[1] or use_b[2]:
                b1_sb = const.tile([1, D], BF16)
                nc.sync.dma_start(b1_sb[:], b1_in[:])
                b2_sb = const.tile([1, D], BF16)
                nc.sync.dma_start(b2_sb[:], b2_in[:])
                invd_sb = const.tile([1, c.SHARD], BF16)
                nc.sync.dma_start(invd_sb[:], invd_in[:])
            gidx_sb = meta_p.tile([128, TOT // 16], I16)
            nc.sync.dma_start(gidx_sb[:], gidx_in[:])
            dstloc_sb = meta_p.tile([128, TOT_TILES], F32)
            nc.sync.dma_start(dstloc_sb[:], dstloc_in[:])
            negdst_sb = meta_p.tile([128, TOT_TILES], F32)
            nc.sync.dma_start(negdst_sb[:], negdst_in[:])

            # per (b, k): tcol base of span, mcol base within batch
            span_base = {}
            tcol0 = 0
            for b in range(c.NBATCH):
                m0 = 0
                for k in range(c.NB):
                    span_base[(b, k)] = (tcol0, m0)
                    tcol0 += int(span_tiles[b, k])
                    m0 += int(span_tiles[b, k])

            def emit_S(Sp, Ap, tcol):
                """Build one-hot S tile for tile column tcol on DVE or ACT."""
                S = Sp.tile([128, 128], BF16, tag="S")
                if (tcol % 10) < c.ACT_TENTHS:
                    a = Ap.tile([128, 128], BF16, tag="a")
                    nc.scalar.activation(
                        a[:], iota_sb[:], ACTF.Abs,
                        bias=negdst_sb[:, tcol:tcol + 1])
                    nc.scalar.activation(
                        S[:], a[:], ACTF.Relu, scale=-1.0, bias=1.0)
                else:
                    nc.vector.tensor_scalar(
                        S[:], iota_sb[:], dstloc_sb[:, tcol:tcol + 1], None,
                        mybir.AluOpType.is_equal)
                return S

            def layer(lid, h_tiles, out_pool, make_next, next_lid):
                """Aggregate layer lid; returns next layer's hs tiles."""
                out_tiles = []
                wr = {k: [] for k in range(c.NB)}    # band -> shard writes
                trig = {}                            # band -> bands triggered
                waited = set()
                with tc.tile_pool(name=f"M{lid}", bufs=2) as Mp, \
                     tc.tile_pool(name=f"S{lid}", bufs=16) as Sp, \
                     tc.tile_pool(name=f"A{lid}", bufs=8) as Ap, \
                     tc.tile_pool(name=f"ag{lid}", bufs=4, space="PSUM") as agp, \
                     tc.tile_pool(name=f"tp{lid}", bufs=2, space="PSUM") as tpp, \
                     tc.tile_pool(name=f"ep{lid}", bufs=4) as epp:
                    for b in range(c.NBATCH):
                        wlo, whi = b * c.WB, min((b + 1) * c.WB, c.NW)
                        Mt = Mp.tile([128, TBMAX, 128], BF16, tag="M")
                        for k in range(c.NB):
                            ntiles = int(span_tiles[b, k])
                            if ntiles == 0:
                                continue
                            tb, mb = span_base[(b, k)]
                            dep = None
                            if k not in waited:
                                dep = ccst["inst"][(lid, k)]
                                waited.add(k)
                            emit_gather(
                                Mt[:, mb:mb + ntiles, :],
                                table_b[lid][k][:],
                                gidx_sb[:, tb * 8:(tb + ntiles) * 8],
                                ntiles * 128, dep_cc=dep)
                        for w in range(wlo, whi):
                            ps = agp.tile([128, D], F32, tag="agg")
                            nmm = int(T[w].sum())
                            nc.tensor.matmul(ps[:], lhsT=ident_sb[:],
                                             rhs=h_tiles[w][:],
                                             start=True, stop=(nmm == 0 and
                                                               not use_b[lid]))
                            mi = 0
                            for k in range(c.NB):
                                tb, mb = span_base[(b, k)]
                                # tiles of (w, k) within the span
                                off = int(T[wlo:w, k].sum())
                                for t in range(int(T[w, k])):
                                    tcol = tb + off + t
                                    mcol = mb + off + t
                                    S = emit_S(Sp, Ap, tcol)
                                    mi += 1
                                    nc.tensor.matmul(
                                        ps[:], lhsT=S[:], rhs=Mt[:, mcol, :],
                                        start=False,
                                        stop=(mi == nmm and not use_b[lid]))
                            if use_b[lid]:
                                bsb = b1_sb if lid == 1 else b2_sb
                                nc.tensor.matmul(
                                    ps[:],
                                    lhsT=invd_sb[0:1, w * 128:(w + 1) * 128],
                                    rhs=bsb[:],
                                    start=False, stop=True)
                            if make_next:
                                z = epp.tile([128, D], BF16, tag="z")
                                nc.scalar.activation(
                                    z[:], ps[:], ACTF.Relu,
                                    scale=dinv_sb[:, w:w + 1])
                                zt_ps = tpp.tile([128, D], BF16, tag="zt")
                                nc.tensor.transpose(zt_ps[:], z[:], ident_sb[:])
                                zT = epp.tile([128, D], BF16, tag="zT")
                                nc.scalar.activation(zT[:], zt_ps[:], ACTF.Copy)
                                h2ps = tpp.tile([128, D], F32, tag="h2")
                                nc.tensor.matmul(h2ps[:], lhsT=zT[:],
                                                 rhs=w2_sb[:],
                                                 start=True, stop=True)
                                ht = out_pool.tile([128, D], BF16, tag="nxt")
                                nc.scalar.activation(
                                    ht[:], h2ps[:], ACTF.Copy,
                                    scale=dinv_sb[:, w:w + 1])
                            else:
                                ht = out_pool.tile([128, D], BF16, tag="nxt")
                                nc.scalar.activation(
                                    ht[:], ps[:], ACTF.Relu,
                                    scale=dinv_sb[:, w:w + 1])
                            for k2, inst in write_shard(next_lid, w, ht):
                                wr[k2].append(inst)
                            out_tiles.append(ht)
                            # trigger next-layer collectives as bands complete
                            for k2 in range(c.NB):
                                if k2 in trig:
                                    continue
                                need_w = math.ceil(((k2 + 1) * c.BROWS) / 128)
                                if w + 1 >= need_w:
                                    trig[k2] = True
                                    emit_collective(next_lid, k2, wr[k2])
                return out_tiles

            with tc.tile_pool(name="hsb2", bufs=c.NW) as hsb2:
                with tc.tile_pool(name="hsb1", bufs=c.NW) as hsb1:
                    # P0: hs1 = dinv * (x @ W1) for own shard
                    h1_tiles = []
                    wr1 = {k: [] for k in range(c.NB)}
                    trig1 = {}
                    with tc.tile_pool(name="p0", bufs=3) as p0, \
                         tc.tile_pool(name="p0ps", bufs=2, space="PSUM") as p0ps:
                        for w in range(c.NW):
                            xt = p0.tile([D, 128], BF16)
                            nc.sync.dma_start(
                                xt[:], xT_in[:, w * 128:(w + 1) * 128])
                            ps = p0ps.tile([128, D], F32, tag="ps")
                            nc.tensor.matmul(ps[:], lhsT=xt[:], rhs=w1_sb[:],
                                             start=True, stop=True)
                            h1t = hsb1.tile([128, D], BF16, tag="h1t")
                            nc.scalar.activation(
                                h1t[:], ps[:], ACTF.Copy,
                                scale=dinv_sb[:, w:w + 1])
                            for k2, inst in write_shard(1, w, h1t):
                                wr1[k2].append(inst)
                            h1_tiles.append(h1t)
                            for k2 in range(c.NB):
                                if k2 in trig1:
                                    continue
                                need_w = math.ceil(((k2 + 1) * c.BROWS) / 128)
                                if w + 1 >= need_w:
                                    trig1[k2] = True
                                    emit_collective(1, k2, wr1[k2])
                    h2_tiles = layer(1, h1_tiles, hsb2, make_next=True,
                                     next_lid=2)
                with tc.tile_pool(name="zsink", bufs=3) as zsink:
                    layer(2, h2_tiles, zsink, make_next=False, next_lid=3)

            # decode
            with tc.tile_pool(name="didx", bufs=1) as didxp, \
                 tc.tile_pool(name="dM", bufs=1) as dMp, \
                 tc.tile_pool(name="dw", bufs=6) as dwp, \
                 tc.tile_pool(name="dout", bufs=1) as doutp:
                ds_sb = didxp.tile([128, TOT_DEC // 16], I16)
                nc.sync.dma_start(ds_sb[:], didx_s_in[:])
                dd_sb = didxp.tile([128, TOT_DEC // 16], I16)
                nc.sync.dma_start(dd_sb[:], didx_d_in[:])
                Ms = dMp.tile([128, TOT_DEC // 128, D], BF16, tag="Ms")
                Md = dMp.tile([128, TOT_DEC // 128, D], BF16, tag="Md")
                res = doutp.tile([128, TOT_DEC // 128], F32)
                waited = set()
                coff = 0
                for g in gorder:
                    ks, kd = g // c.NB, g % c.NB
                    ncols = int(Tdec[g])
                    if ncols == 0:
                        continue
                    dep_s = dep_d = None
                    if ks not in waited:
                        dep_s = ccst["inst"][(3, ks)]
                        waited.add(ks)
                    if kd not in waited:
                        dep_d = ccst["inst"][(3, kd)]
                        waited.add(kd)
                    off16 = coff * 8
                    emit_gather(Ms[:, coff:coff + ncols, :],
                                table_b[3][ks][:],
                                ds_sb[:, off16:off16 + ncols * 8], ncols * 128,
                                dep_cc=dep_s)
                    emit_gather(Md[:, coff:coff + ncols, :],
                                table_b[3][kd][:],
                                dd_sb[:, off16:off16 + ncols * 8], ncols * 128,
                                dep_cc=dep_d)
                    for t in range(ncols):
                        col = coff + t
                        mm = dwp.tile([128, D], F32, tag="mm")
                        nc.vector.tensor_tensor(
                            mm[:], Ms[:, col, :], Md[:, col, :],
                            op=mybir.AluOpType.mult)
                        trash = dwp.tile([128, D], BF16, tag="tr")
                        nc.scalar.activation(
                            trash[:], mm[:], ACTF.Copy,
                            accum_out=res[:, col:col + 1])
                    coff += ncols
                nc.sync.dma_start(dots_out[:], res[:])

    nc.compile()
    return nc


def assemble_output(cfg, meta, results):
    c = cfg
    slot2j = meta["slot2j"]
    out = np.zeros(c.EL, dtype=np.float32)
    for core in range(len(results)):
        d = np.asarray(results[core]["dots"], dtype=np.float32)
        flat = d.T.reshape(-1)             # slot i -> d[i%128, i//128]
        s2j = slot2j[core]
        valid = s2j >= 0
        out[s2j[valid]] = flat[valid]
    return out


def run_pipeline(x, edge_index, edge_label_index, W1, b1, W2, b2,
                 cfg=None, trace=False, tmpdir=None):
    cfg = cfg or DEFAULT
    in_maps, meta = host_prep(cfg, x, edge_index, edge_label_index,
                              W1, b1, W2, b2)
    nc = build_program(cfg, meta)
    res = run_bass_kernel_spmd(nc, in_maps, list(range(cfg.NC)),
                               trace=trace, tmpdir=tmpdir)
    return assemble_output(cfg, meta, res.results), res


def kernel(x, edge_index, edge_label_index, W1, b1, W2, b2):
    out, _ = run_pipeline(x, edge_index, edge_label_index, W1, b1, W2, b2)
    return out
